# revision 1
# baseline (speedup 1.0000x reference)
"""GCN message-passing kernel for 8 Trainium2 NeuronCores.

Model (PyG GCNConv x3 + MLP head + softmax):
    A01 = adjacency + self loops (unit weights), deg = in-degree over A01
    conv(H, W) = D^-1/2 A01 D^-1/2 (H @ W)
    h = x; h = leaky(conv(h, Wg_l)) x3
    y = softmax(leaky(leaky(h @ Wfc1) @ Wfc2))

Key algebraic rewrite: leaky_relu is positively homogeneous, so the
D^-1/2 factors can be pulled out of every leaky() and folded into the
per-layer "message table" G_l:
    G_1 = D^-1/2 (x @ Wg0)
    Ht_{l+1} = leaky(A01 @ G_l)            (pure 0/1 segment-sum!)
    G_{l+1} = D^-1 (Ht_{l+1} @ Wg_l)
    final: z = D^-1/2 leaky(leaky(Ht_4 @ Wfc1) @ Wfc2), out = softmax(z)

Sharding: destination nodes are split into 8 contiguous blocks of 6250
(padded to 6272 = 49 windows of 128). Each layer: every core computes its
G shard (matmul + per-row scale), an AllGather builds the full G table in
DRAM, then each core gathers source rows for its edges (dma_gather,
int16 indices, table split in two <32768-row halves), builds a 0/1
one-hot matrix per 128-edge chunk on the Vector engine (is_equal vs an
iota row), and accumulates  msg^T @ onehot  into a PSUM window on the
TensorEngine.  The flush produces the next layer's activations already
transposed (feat x rows), which is exactly the lhsT layout the next
matmul needs.
"""

import numpy as np

P = 128
N_CORES = 8


def _gw(NW):
    """Windows per gather group."""
    return 7 if NW % 7 == 0 else 1


# --------------------------------------------------------------------------
# Host-side preprocessing: shard edges by destination, pad to fixed chunk
# counts (SPMD requires an identical instruction stream on all cores).
# --------------------------------------------------------------------------
def _preprocess(x, edge_index):
    N, D = x.shape
    assert D == P
    NL = N // N_CORES                      # real nodes per core
    NW = (NL + P - 1) // P                 # windows per core
    NLP = NW * P                           # padded nodes per core
    NGP = N_CORES * NLP                    # padded global nodes
    HALF = 32768                           # int16 gather index limit

    src = np.asarray(edge_index[0], dtype=np.int64)
    dst = np.asarray(edge_index[1], dtype=np.int64)
    loop = np.arange(N, dtype=np.int64)
    src_all = np.concatenate([src, loop])
    dst_all = np.concatenate([dst, loop])

    # in-degree (counts, incl self loops) -- pure index metadata
    deg = np.bincount(dst_all, minlength=N).astype(np.float32)

    # padded global id of each source node
    sowner = src_all // NL
    spid = sowner * NLP + (src_all - sowner * NL)

    owner = dst_all // NL                  # destination owner core
    lid = dst_all - owner * NL             # local dest id on that core
    w = lid // P                           # window
    dr = (lid % P).astype(np.float32)      # one-hot row within window
    half = (spid >= HALF).astype(np.int64)
    srel = np.where(half == 1, spid - HALF, spid)  # idx within its half

    # bucket key: (core, window, half)
    key = (owner * NW + w) * 2 + half
    nbuckets = N_CORES * NW * 2
    order = np.argsort(key, kind="stable")
    key_s = key[order]
    srel_s = srel[order]
    dr_s = dr[order]

    counts = np.bincount(key_s, minlength=nbuckets)
    clo = counts.reshape(-1, 2)[:, 0]
    chi = counts.reshape(-1, 2)[:, 1]
    NCHL = int(np.ceil(clo.max() / P))     # lo chunks per window
    NCHH = int(np.ceil(chi.max() / P))     # hi chunks per window
    CAPL, CAPH = NCHL * P, NCHH * P

    # destination slot of each edge inside the padded per-bucket arrays
    cap = np.where((np.arange(nbuckets) % 2) == 0, CAPL, CAPH)
    base = np.zeros(nbuckets + 1, dtype=np.int64)
    np.cumsum(cap, out=base[1:])
    start = np.zeros(nbuckets, dtype=np.int64)
    start[1:] = np.cumsum(counts)[:-1]
    within = np.arange(len(key_s)) - start[key_s]
    dest = base[key_s] + within

    total_cap = int(base[-1])
    idx_flat = np.zeros(total_cap, dtype=np.int16)
    dr_flat = np.full(total_cap, 200.0, dtype=np.float32)
    idx_flat[dest] = srel_s.astype(np.int16)
    dr_flat[dest] = dr_s

    # per-core views: [NW, 2-half blocks]
    per_core = []
    cap_core = NW * (CAPL + CAPH)
    for c in range(N_CORES):
        seg_i = idx_flat[c * cap_core:(c + 1) * cap_core]
        seg_d = dr_flat[c * cap_core:(c + 1) * cap_core]
        # window-major [NW, CAPL+CAPH]
        seg_i = seg_i.reshape(NW, CAPL + CAPH)
        seg_d = seg_d.reshape(NW, CAPL + CAPH)
        ilo = seg_i[:, :CAPL]              # [NW, CAPL]
        ihi = seg_i[:, CAPL:]
        dlo = seg_d[:, :CAPL]
        dhi = seg_d[:, CAPL:]
        per_core.append((ilo, ihi, dlo, dhi))

    meta = dict(N=N, NL=NL, NW=NW, NLP=NLP, NGP=NGP, HALF=HALF,
                NCHL=NCHL, NCHH=NCHH, deg=deg)
    return per_core, meta


def _wrap_idx_groups(idx_win, ngrp, gw):
    """idx_win: [NW, CAP] int16, window-major edge slots.
    Returns [128, NW*CAP/16] int16 in dma_gather's wrapped layout:
    per gather call (= group of gw windows) logical index j lives at
    [j % 16, j // 16], replicated 8x across the 128 partitions."""
    NW, CAP = idx_win.shape
    cols = []
    for g in range(ngrp):
        block = idx_win[g * gw:(g + 1) * gw].reshape(-1)   # [gw*CAP]
        m = block.reshape(-1, 16).T                        # [16, gw*CAP/16]
        cols.append(np.tile(m, (8, 1)))                    # [128, ...]
    return np.ascontiguousarray(np.concatenate(cols, axis=1))


def _build_core_inputs(x, Ws, per_core, meta):
    """Build the per-core device input dict."""
    N, NL, NW, NLP = meta["N"], meta["NL"], meta["NW"], meta["NLP"]
    NCHL, NCHH = meta["NCHL"], meta["NCHH"]
    deg = meta["deg"]
    GW = _gw(NW)
    ngrp = NW // GW
    Wg0, Wg1, Wg2, Wfc1, Wfc2 = Ws

    iota = np.tile(np.arange(P, dtype=np.float32), (P, 1))
    # Wfc2 [256, 2] -> [128, 4]: cols 0:2 first half of u, 2:4 second half
    Wfc2p = np.concatenate([Wfc2[:P, :], Wfc2[P:, :]], axis=1)
    Wfc2p = np.ascontiguousarray(Wfc2p, dtype=np.float32)

    in_maps = []
    for c in range(N_CORES):
        ilo, ihi, dlo, dhi = per_core[c]
        xs = np.zeros((NLP, P), dtype=np.float32)
        xs[:NL] = x[c * NL:(c + 1) * NL]
        x_t = np.ascontiguousarray(xs.T)                   # [128, NLP]

        degp = np.ones(NLP, dtype=np.float32)
        degp[:NL] = deg[c * NL:(c + 1) * NL]
        deg_t = np.ascontiguousarray(degp.reshape(NW, P).T)  # [128, NW]

        # dstrel: [128, NW*NCH] col = w*NCH + k, row p = edge slot
        drl = np.ascontiguousarray(
            dlo.reshape(NW, NCHL, P).transpose(2, 0, 1).reshape(P, NW * NCHL))
        drh = np.ascontiguousarray(
            dhi.reshape(NW, NCHH, P).transpose(2, 0, 1).reshape(P, NW * NCHH))

        hi_part = {}
        if NCHH:
            hi_part = {"idx_hi": _wrap_idx_groups(ihi, ngrp, GW),
                       "dstrel_hi": drh}
        in_maps.append({
            "x_t": x_t,
            "deg_t": deg_t,
            "idx_lo": _wrap_idx_groups(ilo, ngrp, GW),
            "dstrel_lo": drl,
            **hi_part,
            "iota": iota,
            "Wg0": np.ascontiguousarray(Wg0, dtype=np.float32),
            "Wg1": np.ascontiguousarray(Wg1, dtype=np.float32),
            "Wg2": np.ascontiguousarray(Wg2, dtype=np.float32),
            "Wfc1": np.ascontiguousarray(Wfc1, dtype=np.float32),
            "Wfc2p": Wfc2p,
        })
    return in_maps


# --------------------------------------------------------------------------
# Device program
# --------------------------------------------------------------------------
def _build_bass(meta, mock_cc=False, opts=None):
    opts = opts or {}
    from concourse import bass, bacc, mybir
    import concourse.tile as tile

    NW, NLP, NGP, HALF = meta["NW"], meta["NLP"], meta["NGP"], meta["HALF"]
    NCHL, NCHH = meta["NCHL"], meta["NCHH"]
    GW = _gw(NW)
    NGRP = NW // GW
    GLL = GW * NCHL * P                    # lo idxs per gather call
    GLH = GW * NCHH * P
    f32 = mybir.dt.float32
    bf16 = mybir.dt.bfloat16
    i16 = mybir.dt.int16
    ALL = [list(range(N_CORES))]

    nc = bacc.Bacc("TRN2", target_bir_lowering=False, debug=False,
                   num_devices=N_CORES)

    x_t_d = nc.dram_tensor("x_t", [P, NLP], f32, kind="ExternalInput")
    deg_d = nc.dram_tensor("deg_t", [P, NW], f32, kind="ExternalInput")
    ilo_d = nc.dram_tensor("idx_lo", [P, NW * NCHL * 8], i16, kind="ExternalInput")
    drl_d = nc.dram_tensor("dstrel_lo", [P, NW * NCHL], f32, kind="ExternalInput")
    if NCHH:
        ihi_d = nc.dram_tensor("idx_hi", [P, NW * NCHH * 8], i16,
                               kind="ExternalInput")
        drh_d = nc.dram_tensor("dstrel_hi", [P, NW * NCHH], f32,
                               kind="ExternalInput")
    iota_d = nc.dram_tensor("iota", [P, P], f32, kind="ExternalInput")
    wg_d = [nc.dram_tensor(f"Wg{i}", [P, P], f32, kind="ExternalInput")
            for i in range(3)]
    wfc1_d = nc.dram_tensor("Wfc1", [P, 256], f32, kind="ExternalInput")
    wfc2_d = nc.dram_tensor("Wfc2p", [P, 4], f32, kind="ExternalInput")
    out_d = nc.dram_tensor("out", [NLP, 2], f32, kind="ExternalOutput")

    with tile.TileContext(nc) as tc:
        with (
            tc.tile_pool(name="const", bufs=1) as cpool,
            tc.tile_pool(name="msg", bufs=2) as mpool,
            tc.tile_pool(name="oh", bufs=2) as ohpool,
            tc.tile_pool(name="work", bufs=3) as wpool,
            tc.tile_pool(name="acc", bufs=6, space="PSUM") as ppool,
            tc.tile_pool(name="accy", bufs=2, space="PSUM") as p2pool,
            tc.tile_pool(name="dram", bufs=1, space="DRAM") as dpool,
        ):
            # ---- constants / casts ----
            T_a = cpool.tile([P, NLP], bf16, name="T_a")
            nc.gpsimd.dma_start(out=T_a[:], in_=x_t_d[:])   # f32->bf16 cast
            T_b = cpool.tile([P, NLP], bf16, name="T_b")

            iota_sb = cpool.tile([P, P], bf16, name="iota_sb")
            nc.gpsimd.dma_start(out=iota_sb[:], in_=iota_d[:])
            wg_sb = []
            for i in range(3):
                t = cpool.tile([P, P], bf16, name=f"wg_sb{i}")
                nc.gpsimd.dma_start(out=t[:], in_=wg_d[i][:])
                wg_sb.append(t)
            wfc1_sb = cpool.tile([P, 256], bf16, name="wfc1_sb")
            nc.gpsimd.dma_start(out=wfc1_sb[:], in_=wfc1_d[:])
            wfc2_sb = cpool.tile([P, 4], bf16, name="wfc2_sb")
            nc.gpsimd.dma_start(out=wfc2_sb[:], in_=wfc2_d[:])
            drl_sb = cpool.tile([P, NW * NCHL], bf16, name="drl_sb")
            nc.gpsimd.dma_start(out=drl_sb[:], in_=drl_d[:])  # f32->bf16
            ilo_sb = cpool.tile([P, NW * NCHL * 8], i16, name="ilo_sb")
            nc.sync.dma_start(out=ilo_sb[:], in_=ilo_d[:])
            if NCHH:
                drh_sb = cpool.tile([P, NW * NCHH], bf16, name="drh_sb")
                nc.gpsimd.dma_start(out=drh_sb[:], in_=drh_d[:])
                ihi_sb = cpool.tile([P, NW * NCHH * 8], i16, name="ihi_sb")
                nc.sync.dma_start(out=ihi_sb[:], in_=ihi_d[:])

            deg_sb = cpool.tile([P, NW], f32, name="deg_sb")
            nc.sync.dma_start(out=deg_sb[:], in_=deg_d[:])
            invdeg = cpool.tile([P, NW], f32, name="invdeg")
            nc.vector.reciprocal(invdeg[:], deg_sb[:])
            dinv = cpool.tile([P, NW], f32, name="dinv")
            nc.scalar.sqrt(dinv[:], invdeg[:])

            # NB: collective outputs in Local addr space — Shared
            # scratchpad DMA reads measured ~3x slower on the gather path.
            gfull = [dpool.tile([NGP, P], bf16, name=f"gfull{i}")
                     for i in range(3)]
            gloc = [dpool.tile([NLP, P], bf16, name=f"gloc{i}")
                    for i in range(3)]

            Copy = mybir.ActivationFunctionType.Copy

            def g_production(l, Tsrc):
                scale = dinv if l == 0 else invdeg
                for w in range(NW):
                    ps = ppool.tile([P, P], f32, tag="acc", name="psg")
                    nc.tensor.matmul(ps[:], lhsT=Tsrc[:, w * P:(w + 1) * P],
                                     rhs=wg_sb[l][:], start=True, stop=True)
                    gw_t = wpool.tile([P, P], bf16, tag="gw", name="gw_t")
                    nc.scalar.activation(gw_t[:], ps[:], Copy,
                                         bias=0.0, scale=scale[:, w:w + 1])
                    nc.sync.dma_start(out=gloc[l][w * P:(w + 1) * P, :],
                                      in_=gw_t[:])
                if mock_cc:
                    # single-core timing mock: local copy approximating the
                    # AllGather's local write volume
                    for c in range(N_CORES):
                        nc.sync.dma_start(
                            out=gfull[l][c * NLP:(c + 1) * NLP, :],
                            in_=gloc[l][:])
                else:
                    nc.gpsimd.collective_compute(
                        "AllGather", mybir.AluOpType.bypass,
                        replica_groups=ALL,
                        ins=[gloc[l][:]], outs=[gfull[l][:]])

            def leaky_into(dst_ap, ps):
                t = wpool.tile([P, dst_ap.shape[-1]], f32, tag="lk", name="lkt")
                nc.scalar.activation(t[:], ps[:], Copy, bias=0.0, scale=0.01)
                nc.vector.tensor_tensor(out=dst_ap, in0=ps[:], in1=t[:],
                                        op=mybir.AluOpType.max)


            ohb_const = {}
            if opts.get("skip_ohbuild"):
                GWl = _gw(NW)
                KL = GWl * NCHL
                t = cpool.tile([P, KL * P], bf16, name="ohc_lo")
                nc.vector.tensor_tensor(
                    out=t[:].rearrange("p (k r) -> p k r", r=P),
                    in0=iota_sb[:].unsqueeze(1).to_broadcast([P, KL, P]),
                    in1=drl_sb[:, 0:KL].unsqueeze(2)
                        .to_broadcast([P, KL, P]),
                    op=mybir.AluOpType.is_equal)
                ohb_const["lo"] = t
                if NCHH:
                    KH = GWl * NCHH
                    t2 = cpool.tile([P, KH * P], bf16, name="ohc_hi")
                    nc.vector.tensor_tensor(
                        out=t2[:].rearrange("p (k r) -> p k r", r=P),
                        in0=iota_sb[:].unsqueeze(1).to_broadcast([P, KH, P]),
                        in1=drh_sb[:, 0:KH].unsqueeze(2)
                            .to_broadcast([P, KH, P]),
                        op=mybir.AluOpType.is_equal)
                    ohb_const["hi"] = t2

            def scatter(l, Tdst):
                glo_ap = gfull[l][0:min(HALF, NGP), :]
                ghi_ap = gfull[l][HALF:NGP, :] if NCHH else None
                ntot = NCHL + NCHH
                for g in range(NGRP):
                    mlo = mpool.tile([P, GLL], bf16, tag="mlo", name="mlo")
                    if opts.get("fake_gather"):
                        nc.sync.dma_start(
                            out=mlo[:].rearrange("p (c e) -> p c e", e=P),
                            in_=gfull[l][0:GLL, :].rearrange(
                                "(c p) e -> p c e", p=P))
                    else:
                        nc.gpsimd.dma_gather(
                            out_ap=mlo[:].rearrange("p (c e) -> p c e", e=P),
                            in_ap=glo_ap,
                            idxs_ap=ilo_sb[:, g * (GLL // 16):
                                           (g + 1) * (GLL // 16)],
                            num_idxs=GLL, num_idxs_reg=GLL, elem_size=P,
                            single_packet=False)
                    if NCHH:
                        mhi = mpool.tile([P, GLH], bf16, tag="mhi", name="mhi")
                        if opts.get("fake_gather"):
                            nc.sync.dma_start(
                                out=mhi[:].rearrange("p (c e) -> p c e", e=P),
                                in_=gfull[l][0:GLH, :].rearrange(
                                    "(c p) e -> p c e", p=P))
                        else:
                            nc.gpsimd.dma_gather(
                                out_ap=mhi[:].rearrange("p (c e) -> p c e",
                                                        e=P),
                                in_ap=ghi_ap,
                                idxs_ap=ihi_sb[:, g * (GLH // 16):
                                               (g + 1) * (GLH // 16)],
                                num_idxs=GLH, num_idxs_reg=GLH, elem_size=P,
                                single_packet=False)
                    # batched one-hot build: one DVE op per (group, half)
                    KL = GW * NCHL
                    if opts.get("skip_ohbuild"):
                        ohb_lo = ohb_const["lo"]
                    else:
                        ohb_lo = ohpool.tile([P, KL * P], bf16, tag="ohlo",
                                             name="ohb_lo")
                        nc.vector.tensor_tensor(
                            out=ohb_lo[:].rearrange("p (k r) -> p k r", r=P),
                            in0=iota_sb[:].unsqueeze(1)
                                .to_broadcast([P, KL, P]),
                            in1=drl_sb[:, g * KL:(g + 1) * KL].unsqueeze(2)
                                .to_broadcast([P, KL, P]),
                            op=mybir.AluOpType.is_equal)
                    if NCHH:
                        KH = GW * NCHH
                        if opts.get("skip_ohbuild"):
                            ohb_hi = ohb_const["hi"]
                        else:
                            ohb_hi = ohpool.tile([P, KH * P], bf16,
                                                 tag="ohhi", name="ohb_hi",
                                                 bufs=opts.get("hibufs", 1))
                            nc.vector.tensor_tensor(
                                out=ohb_hi[:].rearrange("p (k r) -> p k r",
                                                        r=P),
                                in0=iota_sb[:].unsqueeze(1)
                                    .to_broadcast([P, KH, P]),
                                in1=drh_sb[:, g * KH:(g + 1) * KH]
                                    .unsqueeze(2).to_broadcast([P, KH, P]),
                                op=mybir.AluOpType.is_equal)
                    for wi in range(GW):
                        w = g * GW + wi
                        ps = ppool.tile([P, P], f32, tag="acc", name="pss")
                        if not opts.get("skip_matmul"):
                            for k in range(ntot):
                                if k < NCHL:
                                    c = wi * NCHL + k
                                    m_ap = mlo[:, c * P:(c + 1) * P]
                                    oh_ap = ohb_lo[:, c * P:(c + 1) * P]
                                else:
                                    c = wi * NCHH + (k - NCHL)
                                    m_ap = mhi[:, c * P:(c + 1) * P]
                                    oh_ap = ohb_hi[:, c * P:(c + 1) * P]
                                nc.tensor.matmul(ps[:], lhsT=m_ap, rhs=oh_ap,
                                                 start=(k == 0),
                                                 stop=(k == ntot - 1))
                            if not opts.get("skip_leaky"):
                                leaky_into(Tdst[:, w * P:(w + 1) * P], ps)

            def head(Tsrc):
                X = mybir.AxisListType.X
                Exp = mybir.ActivationFunctionType.Exp
                for w in range(NW):
                    y1t = []
                    for h in range(2):
                        ps1 = ppool.tile([P, P], f32, tag="acc", name="ps1")
                        nc.tensor.matmul(ps1[:],
                                         lhsT=wfc1_sb[:, h * P:(h + 1) * P],
                                         rhs=Tsrc[:, w * P:(w + 1) * P],
                                         start=True, stop=True)
                        yt = wpool.tile([P, P], bf16, tag=f"y1_{h}",
                                        name="yt")
                        leaky_into(yt[:], ps1)
                        y1t.append(yt)
                    ps2 = p2pool.tile([P, 2], f32, tag="y2", name="ps2")
                    nc.tensor.matmul(ps2[:], lhsT=y1t[0][:],
                                     rhs=wfc2_sb[:, 0:2],
                                     start=True, stop=False)
                    nc.tensor.matmul(ps2[:], lhsT=y1t[1][:],
                                     rhs=wfc2_sb[:, 2:4],
                                     start=False, stop=True)
                    y2 = wpool.tile([P, 2], f32, tag="y2s", name="y2")
                    leaky_into(y2[:], ps2)
                    z = wpool.tile([P, 2], f32, tag="z", name="z")
                    nc.scalar.activation(z[:], y2[:], Copy, bias=0.0,
                                         scale=dinv[:, w:w + 1])
                    negm = wpool.tile([P, 1], f32, tag="m", name="negm")
                    nc.vector.reduce_max(out=negm[:], in_=z[:], axis=X,
                                         negate=True)
                    e = wpool.tile([P, 2], f32, tag="e", name="e")
                    nc.scalar.activation(e[:], z[:], Exp,
                                         bias=negm[:, 0:1], scale=1.0)
                    s = wpool.tile([P, 1], f32, tag="s", name="s")
                    nc.vector.reduce_sum(out=s[:], in_=e[:], axis=X)
                    rs = wpool.tile([P, 1], f32, tag="rs", name="rs")
                    nc.vector.reciprocal(rs[:], s[:])
                    o = wpool.tile([P, 2], f32, tag="o", name="o")
                    nc.vector.tensor_scalar(out=o[:], in0=e[:],
                                            scalar1=rs[:, 0:1], scalar2=None,
                                            op0=mybir.AluOpType.mult)
                    nc.sync.dma_start(out=out_d[w * P:(w + 1) * P, :],
                                      in_=o[:])

            if opts.get("skip_scatter"):
                g_production(0, T_a)
                g_production(1, T_a)
                g_production(2, T_a)
                head(T_a)
            else:
                g_production(0, T_a)
                scatter(0, T_b)
                g_production(1, T_b)
                scatter(1, T_a)
                g_production(2, T_a)
                scatter(2, T_b)
                head(T_b)

    nc.compile()
    return nc


# --------------------------------------------------------------------------
# Entry point
# --------------------------------------------------------------------------
LAST_RESULT = None
LAST_NC = None
LAST_IN_MAPS = None
LAST_META = None


def kernel(x, edge_index, Wg0, Wg1, Wg2, Wfc1, Wfc2):
    from concourse.bass_utils import run_bass_kernel_spmd

    global LAST_RESULT, LAST_NC, LAST_IN_MAPS, LAST_META
    x = np.asarray(x)
    edge_index = np.asarray(edge_index)
    per_core, meta = _preprocess(x, edge_index)
    in_maps = _build_core_inputs(
        x, (np.asarray(Wg0), np.asarray(Wg1), np.asarray(Wg2),
            np.asarray(Wfc1), np.asarray(Wfc2)), per_core, meta)
    nc = _build_bass(meta)
    LAST_NC, LAST_IN_MAPS, LAST_META = nc, in_maps, meta
    res = run_bass_kernel_spmd(nc, in_maps, core_ids=list(range(N_CORES)))
    LAST_RESULT = res
    NL = meta["NL"]
    out = np.concatenate([res.results[c]["out"][:NL] for c in range(N_CORES)],
                         axis=0)
    return out.astype(np.float32)



# revision 3
# speedup vs baseline: 3.1490x; 3.1490x over previous
"""GCN message-passing kernel for 8 Trainium2 NeuronCores.

Model (PyG GCNConv x3 + MLP head + softmax):
    A01 = adjacency + self loops (unit weights), deg = in-degree over A01
    conv(H, W) = D^-1/2 A01 D^-1/2 (H @ W)
    h = x; h = leaky(conv(h, Wg_l)) x3
    y = softmax(leaky(leaky(h @ Wfc1) @ Wfc2))

Key algebraic rewrite: leaky_relu is positively homogeneous, so the
D^-1/2 factors can be pulled out of every leaky() and folded into the
per-layer "message table" G_l:
    G_1 = D^-1/2 (x @ Wg0)
    Ht_{l+1} = leaky(A01 @ G_l)            (pure 0/1 segment-sum!)
    G_{l+1} = D^-1 (Ht_{l+1} @ Wg_l)
    final: z = D^-1/2 leaky(leaky(Ht_4 @ Wfc1) @ Wfc2), out = softmax(z)

Sharding: destination nodes are split into 8 contiguous blocks of 6250
(padded to 6272 = 49 windows of 128). Each layer: every core computes its
G shard (matmul + per-row scale), an AllGather builds the full G table in
DRAM, then each core gathers source rows for its edges (dma_gather,
int16 indices, table split in two <32768-row halves), builds a 0/1
one-hot matrix per 128-edge chunk on the Vector engine (is_equal vs an
iota row), and accumulates  msg^T @ onehot  into a PSUM window on the
TensorEngine.  The flush produces the next layer's activations already
transposed (feat x rows), which is exactly the lhsT layout the next
matmul needs.
"""

import numpy as np

P = 128
N_CORES = 8


def _gw(NW):
    """Windows per gather group."""
    return 7 if NW % 7 == 0 else 1


# --------------------------------------------------------------------------
# Host-side preprocessing: shard edges by destination, pad to fixed chunk
# counts (SPMD requires an identical instruction stream on all cores).
# --------------------------------------------------------------------------
def _preprocess(x, edge_index):
    N, D = x.shape
    assert D == P
    NL = N // N_CORES                      # real nodes per core
    NW = (NL + P - 1) // P                 # windows per core
    NLP = NW * P                           # padded nodes per core
    NGP = N_CORES * NLP                    # padded global nodes
    NWA = (NW + 1) // 2                    # windows in half-shard A
    HA, HB = NWA * P, NLP - NWA * P        # rows per core in each half

    src = np.asarray(edge_index[0], dtype=np.int64)
    dst = np.asarray(edge_index[1], dtype=np.int64)

    # in-degree incl the self loop (A + I); self loops are NOT gathered --
    # their contribution is added as an identity matmul from local G.
    deg = (np.bincount(dst, minlength=N) + 1).astype(np.float32)

    # source nodes live in one of two AllGather half-tables: table A holds
    # every core's first NWA windows (rows sowner*HA + lrow), table B the
    # rest. Both tables stay under the 32768-row int16 gather-index limit.
    sowner = src // NL
    lrow = src - sowner * NL
    half = (lrow >= HA).astype(np.int64)
    srel = np.where(half == 1, sowner * HB + (lrow - HA),
                    sowner * HA + lrow)
    assert N_CORES * HA <= 32768 and N_CORES * HB <= 32768

    owner = dst // NL                      # destination owner core
    lid = dst - owner * NL                 # local dest id on that core
    w = lid // P                           # window
    dr = (lid % P).astype(np.float32)      # one-hot row within window

    # bucket key: (core, window, half)
    key = (owner * NW + w) * 2 + half
    nbuckets = N_CORES * NW * 2
    order = np.argsort(key, kind="stable")
    key_s = key[order]
    srel_s = srel[order]
    dr_s = dr[order]

    counts = np.bincount(key_s, minlength=nbuckets)
    cnt3 = counts.reshape(N_CORES, NW, 2)
    # SPMD: num_idxs_reg must be an identical immediate on all cores --
    # per (window, half) the gathered count is the max across cores.
    cntmax = cnt3.max(axis=0)              # [NW, 2]
    nchw = np.ceil(cntmax / P).astype(np.int64)   # [NW, 2] chunks
    capw = nchw * P                        # [NW, 2] slot capacity

    # per-core bucket base offsets (window-major, lo block then hi block)
    capflat = capw.reshape(-1)             # [NW*2]
    offflat = np.zeros(NW * 2 + 1, dtype=np.int64)
    np.cumsum(capflat, out=offflat[1:])
    cap_core = int(offflat[-1])
    base = np.empty(nbuckets, dtype=np.int64)
    for c in range(N_CORES):
        base[c * NW * 2:(c + 1) * NW * 2] = c * cap_core + offflat[:-1]

    start = np.zeros(nbuckets, dtype=np.int64)
    start[1:] = np.cumsum(counts)[:-1]
    within = np.arange(len(key_s)) - start[key_s]
    dest = base[key_s] + within

    total_cap = N_CORES * cap_core
    idx_flat = np.full(total_cap, -1, dtype=np.int16)
    dr_flat = np.full(total_cap, 200.0, dtype=np.float32)
    idx_flat[dest] = srel_s.astype(np.int16)
    dr_flat[dest] = dr_s
    # pad with dummy-valid index 0 up to cntmax; tail stays -1 (skipped)
    cm = cntmax.reshape(-1)
    for c in range(N_CORES):
        for b in range(NW * 2):
            lo = int(cnt3.reshape(N_CORES, -1)[c, b])
            hi = int(cm[b])
            if hi > lo:
                p0 = c * cap_core + offflat[b]
                idx_flat[p0 + lo:p0 + hi] = 0

    per_core = [(idx_flat[c * cap_core:(c + 1) * cap_core],
                 dr_flat[c * cap_core:(c + 1) * cap_core])
                for c in range(N_CORES)]

    meta = dict(N=N, NL=NL, NW=NW, NLP=NLP, NGP=NGP, NWA=NWA,
                HA=HA, HB=HB, deg=deg, cntmax=cntmax, nchw=nchw, capw=capw,
                offflat=offflat, cap_core=cap_core)
    return per_core, meta


def _wrap_blocks(flat, offflat):
    """flat: [cap_core] int16 slot indices. Wrap each (window,half) block
    separately into dma_gather's layout ([j%16, j//16], replicated to 128
    partitions), concatenated along columns."""
    cols = []
    for b in range(len(offflat) - 1):
        blk = flat[offflat[b]:offflat[b + 1]]
        if len(blk) == 0:
            continue
        m = blk.reshape(-1, 16).T              # [16, cap/16]
        cols.append(np.tile(m, (8, 1)))        # [128, cap/16]
    return np.ascontiguousarray(np.concatenate(cols, axis=1))


def _chunk_major(flat, offflat):
    """dr values per slot -> [128, total_chunks] (column per chunk, row =
    slot within chunk), blocks in (window, half) order."""
    cols = []
    for b in range(len(offflat) - 1):
        blk = flat[offflat[b]:offflat[b + 1]]
        if len(blk) == 0:
            continue
        cols.append(blk.reshape(-1, P).T)      # [128, nch_b]
    return np.ascontiguousarray(np.concatenate(cols, axis=1))


def _build_core_inputs(x, Ws, per_core, meta):
    """Per-core device input dict (bf16 pre-cast on host so the device
    needs no SWDGE casting DMAs)."""
    import ml_dtypes
    bf16 = ml_dtypes.bfloat16
    N, NL, NW, NLP = meta["N"], meta["NL"], meta["NW"], meta["NLP"]
    deg, offflat = meta["deg"], meta["offflat"]
    Wg0, Wg1, Wg2, Wfc1, Wfc2 = Ws

    iota = np.tile(np.arange(P, dtype=np.float32), (P, 1)).astype(bf16)
    ident = np.eye(P, dtype=np.float32).astype(bf16)
    Wfc2p = np.concatenate([Wfc2[:P, :], Wfc2[P:, :]], axis=1)
    Wfc2p = np.ascontiguousarray(Wfc2p).astype(bf16)

    in_maps = []
    for c in range(N_CORES):
        iflat, dflat = per_core[c]
        xs = np.zeros((NLP, P), dtype=np.float32)
        xs[:NL] = x[c * NL:(c + 1) * NL]
        x_t = np.ascontiguousarray(xs.T).astype(bf16)      # [128, NLP]

        degp = np.ones(NLP, dtype=np.float32)
        degp[:NL] = deg[c * NL:(c + 1) * NL]
        deg_t = np.ascontiguousarray(degp.reshape(NW, P).T)  # [128, NW]

        in_maps.append({
            "x_t": x_t,
            "deg_t": deg_t,
            "idx": _wrap_blocks(iflat, offflat),
            "dstrel": _chunk_major(dflat, offflat).astype(bf16),
            "iota": iota,
            "ident": ident,
            "Wg0": np.ascontiguousarray(Wg0).astype(bf16),
            "Wg1": np.ascontiguousarray(Wg1).astype(bf16),
            "Wg2": np.ascontiguousarray(Wg2).astype(bf16),
            "Wfc1": np.ascontiguousarray(Wfc1).astype(bf16),
            "Wfc2p": Wfc2p,
        })
    return in_maps


# --------------------------------------------------------------------------
# Device program
# --------------------------------------------------------------------------
def _build_bass(meta, mock_cc=False, opts=None):
    opts = opts or {}
    from concourse import bass, bacc, mybir
    import concourse.tile as tile

    NW, NLP, NGP = meta["NW"], meta["NLP"], meta["NGP"]
    NWA, HA, HB = meta["NWA"], meta["HA"], meta["HB"]
    cntmax = meta["cntmax"]
    nchw = meta["nchw"]                    # [NW, 2] chunks per window/half
    capw = meta["capw"]                    # = nchw * P
    offflat = meta["offflat"]              # slot offsets per (w, half)
    cap_core = meta["cap_core"]
    GW = _gw(NW)
    NGRP = NW // GW
    f32 = mybir.dt.float32
    bf16 = mybir.dt.bfloat16
    i16 = mybir.dt.int16
    ALL = [list(range(N_CORES))]

    # per-window chunk-column offsets into the dstrel/chunk-major layout,
    # and slot offsets within a group's unified message tile
    nch_flat = nchw.reshape(-1)            # [NW*2]
    choff = np.zeros(NW * 2 + 1, dtype=np.int64)
    np.cumsum(nch_flat, out=choff[1:])
    NCH_TOT = int(choff[-1])
    # group g covers windows [g*GW, (g+1)*GW); its tile holds those
    # windows' lo+hi chunks contiguously in (w, half) order
    grp_nch = [int(choff[(g * GW + GW) * 2] - choff[g * GW * 2])
               for g in range(NGRP)]
    KG_MAX = max(grp_nch)

    NQ = opts.get("nq", 4)                 # SWDGE queues for gathers
    nc = bacc.Bacc("TRN2", target_bir_lowering=False, debug=False,
                   num_devices=N_CORES, num_swdge_queues=NQ)

    x_t_d = nc.dram_tensor("x_t", [P, NLP], bf16, kind="ExternalInput")
    deg_d = nc.dram_tensor("deg_t", [P, NW], f32, kind="ExternalInput")
    idx_d = nc.dram_tensor("idx", [P, cap_core // 16], i16,
                           kind="ExternalInput")
    dr_d = nc.dram_tensor("dstrel", [P, NCH_TOT], bf16,
                          kind="ExternalInput")
    iota_d = nc.dram_tensor("iota", [P, P], bf16, kind="ExternalInput")
    ident_d = nc.dram_tensor("ident", [P, P], bf16, kind="ExternalInput")
    wg_d = [nc.dram_tensor(f"Wg{i}", [P, P], bf16, kind="ExternalInput")
            for i in range(3)]
    wfc1_d = nc.dram_tensor("Wfc1", [P, 256], bf16, kind="ExternalInput")
    wfc2_d = nc.dram_tensor("Wfc2p", [P, 4], bf16, kind="ExternalInput")
    out_d = nc.dram_tensor("out", [NLP, 2], f32, kind="ExternalOutput")

    with tile.TileContext(nc) as tc:
        with (
            tc.tile_pool(name="const", bufs=1) as cpool,
            tc.tile_pool(name="msg", bufs=2) as mpool,
            tc.tile_pool(name="oh", bufs=2) as ohpool,
            tc.tile_pool(name="work", bufs=3) as wpool,
            tc.tile_pool(name="acc", bufs=4, space="PSUM") as ppool,
            tc.tile_pool(name="accy", bufs=1, space="PSUM") as p2pool,
            tc.tile_pool(name="dram", bufs=1, space="DRAM") as dpool,
        ):
            # ---- constants (pre-cast bf16 on host; NO Pool/SWDGE DMAs
            # here: gathers must be the only Pool DMAs so their DMASW sem
            # lanes stay queue-consistent) ----
            T_a = cpool.tile([P, NLP], bf16, name="T_a")
            nc.sync.dma_start(out=T_a[:], in_=x_t_d[:])
            T_b = T_a
            if not opts.get("skip_scatter") and not opts.get("skip_matmul"):
                T_b = cpool.tile([P, NLP], bf16, name="T_b")

            iota_sb = cpool.tile([P, P], bf16, name="iota_sb")
            nc.sync.dma_start(out=iota_sb[:], in_=iota_d[:])
            ident_sb = cpool.tile([P, P], bf16, name="ident_sb")
            nc.sync.dma_start(out=ident_sb[:], in_=ident_d[:])
            wg_sb = []
            for i in range(3):
                t = cpool.tile([P, P], bf16, name=f"wg_sb{i}")
                nc.sync.dma_start(out=t[:], in_=wg_d[i][:])
                wg_sb.append(t)
            wfc1_sb = cpool.tile([P, 256], bf16, name="wfc1_sb")
            nc.sync.dma_start(out=wfc1_sb[:], in_=wfc1_d[:])
            wfc2_sb = cpool.tile([P, 4], bf16, name="wfc2_sb")
            nc.sync.dma_start(out=wfc2_sb[:], in_=wfc2_d[:])
            dr_sb = cpool.tile([P, NCH_TOT], bf16, name="dr_sb")
            nc.sync.dma_start(out=dr_sb[:], in_=dr_d[:])
            idx_sb = cpool.tile([P, cap_core // 16], i16, name="idx_sb")
            nc.sync.dma_start(out=idx_sb[:], in_=idx_d[:])
            # local message-table mirror (layer-current G shard), used for
            # the self-loop identity contribution
            G_sb = cpool.tile([P, NLP], bf16, name="G_sb")

            deg_sb = cpool.tile([P, NW], f32, name="deg_sb")
            nc.sync.dma_start(out=deg_sb[:], in_=deg_d[:])
            invdeg = cpool.tile([P, NW], f32, name="invdeg")
            nc.vector.reciprocal(invdeg[:], deg_sb[:])
            dinv = cpool.tile([P, NW], f32, name="dinv")
            nc.scalar.sqrt(dinv[:], invdeg[:])

            # NB: collective outputs in Local addr space — Shared
            # scratchpad DMA reads measured ~3x slower on the gather path.
            ag_shared = opts.get("ag_shared")            # AG->Shared, copy->Local
            ag_shared_direct = opts.get("ag_shared_direct")  # AG->Shared, gather Shared
            gfullA = [dpool.tile([N_CORES * HA, P], bf16, name=f"gfa{i}")
                      for i in range(3)]
            gfullB = [dpool.tile([N_CORES * HB, P], bf16, name=f"gfb{i}")
                      for i in range(3)]
            gloc = [dpool.tile([NLP, P], bf16, name=f"gloc{i}")
                    for i in range(3)]

            Copy = mybir.ActivationFunctionType.Copy

            def g_production(l, Tsrc, rep=0):
                scale = dinv if l == 0 else invdeg
                for w in range(NW):
                    ps = ppool.tile([P, P], f32, tag="acc", name="psg")
                    nc.tensor.matmul(ps[:], lhsT=Tsrc[:, w * P:(w + 1) * P],
                                     rhs=wg_sb[l][:], start=True, stop=True)
                    gw_t = G_sb[:, w * P:(w + 1) * P]
                    nc.scalar.activation(gw_t, ps[:], Copy,
                                         bias=0.0, scale=scale[:, w:w + 1])
                    nc.sync.dma_start(out=gloc[l][w * P:(w + 1) * P, :],
                                      in_=gw_t)
                    if not mock_cc and w == NWA - 1:
                        # half-shard A complete: AllGather it while the
                        # remaining windows are still being produced
                        nc.gpsimd.collective_compute(
                            "AllGather", mybir.AluOpType.bypass,
                            replica_groups=ALL,
                            ins=[gloc[l][0:HA, :]], outs=[gfullA[l][:]])
                if not mock_cc:
                    nc.gpsimd.collective_compute(
                        "AllGather", mybir.AluOpType.bypass,
                        replica_groups=ALL,
                        ins=[gloc[l][HA:NLP, :]], outs=[gfullB[l][:]])
                if mock_cc:
                    # single-core timing mock: local copy approximating the
                    # AllGather's local write volume
                    for c in range(N_CORES):
                        nc.sync.dma_start(
                            out=gfullA[l][c * HA:(c + 1) * HA, :],
                            in_=gloc[l][0:HA, :])
                        nc.sync.dma_start(
                            out=gfullB[l][c * HB:(c + 1) * HB, :],
                            in_=gloc[l][HA:NLP, :])
                else:
                    pass  # split AllGathers are issued inside the window
                          # loop (A right after window NWA-1) for overlap

            Lrelu = mybir.ActivationFunctionType.Lrelu

            def leaky_into(dst_ap, ps_ap, scale=1.0):
                # leaky_relu on the Act engine; positive homogeneity lets a
                # positive per-row scale fold into the same op. (no_lrelu
                # falls back to copy+max for the sim, which lacks Lrelu.)
                if opts.get("no_lrelu"):
                    assert isinstance(scale, float) and scale == 1.0
                    t = wpool.tile([P, max(2 * NW, 4 * P)], f32, tag="lk",
                                   name="lkt")
                    tt = t[:, 0:ps_ap.shape[-1]]
                    nc.scalar.activation(tt, ps_ap, Copy, bias=0.0,
                                         scale=0.01)
                    nc.vector.tensor_tensor(out=dst_ap, in0=ps_ap, in1=tt,
                                            op=mybir.AluOpType.max)
                    return
                nc.scalar.activation(dst_ap, ps_ap, Lrelu, bias=0.0,
                                     scale=scale, alpha=0.01)


            # Persistent double-buffered unified message tiles (lo+hi
            # chunks of a whole group). Pad slots are never written by the
            # gathers (negative-idx skip) so zero them once.
            m_bufs = []
            for b in range(2 if not opts.get("skip_scatter") else 0):
                t = cpool.tile([P, KG_MAX * P], bf16, name=f"m_b{b}")
                nc.vector.memset(t[:], 0.0)
                m_bufs.append(t)
            qctr = [0]

            ohb_const = None
            if opts.get("skip_ohbuild"):
                ohb_const = cpool.tile([P, KG_MAX * P], bf16, name="ohc")
                nc.vector.tensor_tensor(
                    out=ohb_const[:].rearrange("p (k r) -> p k r", r=P),
                    in0=iota_sb[:].unsqueeze(1).to_broadcast([P, KG_MAX, P]),
                    in1=dr_sb[:, 0:KG_MAX].unsqueeze(2)
                        .to_broadcast([P, KG_MAX, P]),
                    op=mybir.AluOpType.is_equal)

            def scatter(l, Tdst, rep=0):
                glo_ap = gfullA[l][:]
                ghi_ap = gfullB[l][:] if HB > 0 else None
                for g in range(NGRP):
                    mb = m_bufs[g % 2]
                    w_lo = g * GW
                    ch_base = int(choff[w_lo * 2])     # first chunk col
                    KG = grp_nch[g]
                    if opts.get("fake_gather"):
                        nc.sync.dma_start(
                            out=mb[:, 0:KG * P].rearrange(
                                "p (c e) -> p c e", e=P),
                            in_=gfullA[l][0:KG * P, :].rearrange(
                                "(c p) e -> p c e", p=P))
                    else:
                        # per-(window,half) gathers, round-robin over NQ
                        # SWDGE queues; -1 tails are skipped by HW, the
                        # immediate count = max across cores.
                        for wi in range(GW):
                            w = w_lo + wi
                            for h in range(2):
                                cap = int(capw[w, h])
                                if cap == 0:
                                    continue
                                slot0 = int(offflat[2 * w + h]
                                            - offflat[2 * w_lo])
                                nc.gpsimd.dma_gather(
                                    out_ap=mb[:, slot0:slot0 + cap]
                                        .rearrange("p (c e) -> p c e", e=P),
                                    in_ap=glo_ap if h == 0 else ghi_ap,
                                    idxs_ap=idx_sb[
                                        :, int(offflat[2 * w + h]) // 16:
                                        int(offflat[2 * w + h + 1]) // 16],
                                    num_idxs=cap,
                                    num_idxs_reg=int(cntmax[w, h]),
                                    elem_size=P, single_packet=False,
                                    queue_num=qctr[0] % NQ)
                                qctr[0] += 1
                    # batched one-hot build: one DVE op per group
                    if opts.get("skip_ohbuild"):
                        ohb = ohb_const
                    else:
                        ohb = ohpool.tile([P, KG_MAX * P], bf16, tag="ohb",
                                          name="ohb")
                        nc.vector.tensor_tensor(
                            out=ohb[:, 0:KG * P].rearrange(
                                "p (k r) -> p k r", r=P),
                            in0=iota_sb[:].unsqueeze(1)
                                .to_broadcast([P, KG, P]),
                            in1=dr_sb[:, ch_base:ch_base + KG].unsqueeze(2)
                                .to_broadcast([P, KG, P]),
                            op=mybir.AluOpType.is_equal)
                    for wi in range(GW):
                        w = w_lo + wi
                        ps = ppool.tile([P, P], f32, tag="acc", name="pss")
                        if not opts.get("skip_matmul"):
                            # self-loop: out_w += G_local[w]^T
                            nc.tensor.matmul(ps[:],
                                             lhsT=G_sb[:, w * P:(w + 1) * P],
                                             rhs=ident_sb[:],
                                             start=True, stop=False)
                            c0 = int(choff[2 * w]) - ch_base
                            c1 = int(choff[2 * w + 2]) - ch_base
                            for c in range(c0, c1):
                                nc.tensor.matmul(
                                    ps[:],
                                    lhsT=mb[:, c * P:(c + 1) * P],
                                    rhs=ohb[:, c * P:(c + 1) * P],
                                    start=False, stop=(c == c1 - 1))
                            if not opts.get("skip_leaky"):
                                leaky_into(Tdst[:, w * P:(w + 1) * P], ps)

            def head(Tsrc):
                if opts.get("skip_head"):
                    o0 = wpool.tile([P, 2], f32, tag="o", name="o0")
                    nc.scalar.activation(o0[:], Tsrc[:, 0:2], Copy,
                                         bias=0.0, scale=1.0)
                    nc.sync.dma_start(out=out_d[0:P, :], in_=o0[:])
                    return
                Sigmoid = mybir.ActivationFunctionType.Sigmoid
                WB = 4
                ps2b = p2pool.tile([P, 2 * NW], f32, tag="y2b", name="ps2b")
                w0 = 0
                while w0 < NW:
                    wn = min(WB, NW - w0)
                    cols = wn * P
                    y1t = []
                    for h in range(2):
                        ps1 = ppool.tile([P, WB * P], f32, tag="acch",
                                         name="ps1", bufs=2)
                        nc.tensor.matmul(ps1[:, 0:cols],
                                         lhsT=wfc1_sb[:, h * P:(h + 1) * P],
                                         rhs=Tsrc[:, w0 * P:w0 * P + cols],
                                         start=True, stop=True)
                        yt = wpool.tile([P, WB * P], bf16, tag=f"y1_{h}",
                                        name="yt")
                        leaky_into(yt[:, 0:cols], ps1[:, 0:cols])
                        y1t.append(yt)
                    for wi in range(wn):
                        w = w0 + wi
                        nc.tensor.matmul(ps2b[:, 2 * w:2 * w + 2],
                                         lhsT=y1t[0][:, wi * P:(wi + 1) * P],
                                         rhs=wfc2_sb[:, 0:2],
                                         start=True, stop=False)
                        nc.tensor.matmul(ps2b[:, 2 * w:2 * w + 2],
                                         lhsT=y1t[1][:, wi * P:(wi + 1) * P],
                                         rhs=wfc2_sb[:, 2:4],
                                         start=False, stop=True)
                    w0 += wn
                # epilogue (one pass over all windows):
                # softmax over 2 classes == sigmoid of the scaled logit diff
                z = wpool.tile([P, 2 * NW], f32, tag="z", name="z")
                leaky_into(z[:], ps2b[:])
                z3 = z[:].rearrange("p (w c) -> p w c", c=2)
                zd = wpool.tile([P, NW], f32, tag="zd", name="zd")
                nc.vector.tensor_tensor(out=zd[:], in0=z3[:, :, 0],
                                        in1=z3[:, :, 1],
                                        op=mybir.AluOpType.subtract)
                d2 = wpool.tile([P, NW], f32, tag="d2", name="d2")
                nc.vector.tensor_tensor(out=d2[:], in0=zd[:], in1=dinv[:],
                                        op=mybir.AluOpType.mult)
                ob = wpool.tile([P, 2 * NW], f32, tag="ob", name="ob")
                o3 = ob[:].rearrange("p (w c) -> p w c", c=2)
                nc.scalar.activation(o3[:, :, 0], d2[:], Sigmoid,
                                     bias=0.0, scale=1.0)
                nc.scalar.activation(o3[:, :, 1], d2[:], Sigmoid,
                                     bias=0.0, scale=-1.0)
                nc.sync.dma_start(
                    out=out_d[:].rearrange("(w p) c -> p w c", p=P),
                    in_=ob[:].rearrange("p (w c) -> p w c", c=2))

            for _rep in range(opts.get("repeat", 1)):
                if opts.get("skip_scatter"):
                    g_production(0, T_a, _rep)
                    g_production(1, T_a, _rep)
                    g_production(2, T_a, _rep)
                    head(T_a)
                else:
                    g_production(0, T_a, _rep)
                    scatter(0, T_b, _rep)
                    g_production(1, T_b, _rep)
                    scatter(1, T_a, _rep)
                    g_production(2, T_a, _rep)
                    scatter(2, T_b, _rep)
                    head(T_b)

    nc.compile()
    return nc


# --------------------------------------------------------------------------
# Entry point
# --------------------------------------------------------------------------
LAST_RESULT = None
LAST_NC = None
LAST_IN_MAPS = None
LAST_META = None


def kernel(x, edge_index, Wg0, Wg1, Wg2, Wfc1, Wfc2):
    from concourse.bass_utils import run_bass_kernel_spmd

    global LAST_RESULT, LAST_NC, LAST_IN_MAPS, LAST_META
    x = np.asarray(x)
    edge_index = np.asarray(edge_index)
    per_core, meta = _preprocess(x, edge_index)
    in_maps = _build_core_inputs(
        x, (np.asarray(Wg0), np.asarray(Wg1), np.asarray(Wg2),
            np.asarray(Wfc1), np.asarray(Wfc2)), per_core, meta)
    nc = _build_bass(meta)
    LAST_NC, LAST_IN_MAPS, LAST_META = nc, in_maps, meta
    res = run_bass_kernel_spmd(nc, in_maps, core_ids=list(range(N_CORES)))
    LAST_RESULT = res
    NL = meta["NL"]
    out = np.concatenate([res.results[c]["out"][:NL] for c in range(N_CORES)],
                         axis=0)
    return out.astype(np.float32)

